# revision 1
# baseline (speedup 1.0000x reference)
"""ColBERT in-batch-negative loss on 8 Trainium2 NeuronCores.

Strategy: shard the C=128 doc candidates across 8 cores (16 docs each),
replicate the queries. Each core computes its [64, 16] block of the score
matrix (MaxSim over doc tokens, sum over query tokens) plus per-row partial
softmax stats (row max m, sum of exp(k*(raw-m)), and the local c=0 column).
The host combines the 8 partial logsumexps and the label-0 column into the
scalar mean cross-entropy loss (the distributed-softmax all-reduce done at
gather time).

Device kernel per core:
  - one DMA in: [qT | docT_shard | block-ones] as one [128, 4100] f32r tensor
  - 64 f32r matmuls (N=512) computing late = q @ docT for 16 groups of 4
    queries (stationary = 128 cols of qT = 4 queries x 32 tokens)
  - MaxSim d-reduction split across engines: NA groups reduced directly from
    PSUM on the Vector engine; the rest are copied PSUM->SBUF(fp16) by the
    Scalar engine and max-reduced by a Vector-engine fp16 pairwise tree
  - a tiny fp32 ones-matmul sums over the 32 query tokens per query
  - row max / exp / sum-of-exp on [4, 256], one DMA out of [4, 48] stats

Toolchain constraint: this walrus build allows only ONE semaphore wait per
instruction, so the kernel keeps a single-writer/single-reader-engine
discipline everywhere and the Tile tail drain is split into one drain per
processor (SplitDrainTileContext).
"""

import sys

sys.path.insert(0, "/opt/trn_rl_repo")

import numpy as np

import bass_rust
import concourse.bass as bass
import concourse.mybir as mybir
from concourse.tile import TileContext
from concourse.bass_utils import run_bass_kernel_spmd

f32 = mybir.dt.float32
f32r = mybir.dt.float32r
fp16 = mybir.dt.float16

N_CORES = 8
B, SQ, H = 64, 32, 128
C, SD = 128, 128
C_LOC = C // N_CORES          # 16 docs per core
TEMPERATURE = 0.05
K_SCALE = 1.0 / (SQ * TEMPERATURE)   # 0.625

G = 16                        # groups of 4 queries; 4*32 = 128 partitions
NA = 4                        # groups max-reduced directly on DVE from PSUM
A_GROUPS = (0, 5, 10, 15)
B_GROUPS = tuple(g for g in range(G) if g not in A_GROUPS)
# maxsim column block position for each group (A blocks first, then B in
# stream order; B batches of 4 need contiguous blocks for the tree output)
POS = {g: i for i, g in enumerate(A_GROUPS + B_GROUPS)}

QT_COLS = B * SQ              # 2048
DT_COLS = C_LOC * SD          # 2048
IN_COLS = QT_COLS + DT_COLS + 4

_STATE = {}
LAST_RESULTS = None


def _round_f32r(x: np.ndarray) -> np.ndarray:
    """Round fp32 to fp32r (11-bit mantissa) — the PE requires pre-rounded input."""
    u = np.ascontiguousarray(x).view(np.uint32)
    r = ((u >> 12) + ((u >> 11) & 1)) << 12
    return r.astype(np.uint32).view(np.float32)


class SplitDrainTileContext(TileContext):
    """Tail drain needs one wait per used proc but instructions only hold one
    sync wait on this toolchain — emit one SP drain per proc."""

    def _drain_and_barrier(self, tick_clock, wait_clock):
        n = bass_rust.N_PROCS
        full = [tick_clock.global_clock.peek_next(i) - 1 for i in range(n)]
        for idx, v in enumerate(full):
            if v <= 0:
                continue
            part = [v if i == idx else 0 for i in range(n)]
            d = self.nc.sync.drain()
            wait_clock.add_sem_waits(
                d.ins, bass_rust.ScopedClock({None: bass_rust.VectorClock(part)})
            )
        self.nc.all_engine_barrier()
        assert self.sems is not None
        popped = self.nc._tile_sem_poison_stack.pop()
        assert popped is self._sem_poison
        self.nc.clear_and_free_semaphores(list(self.sems.allocated().values()))
        self.nc.all_engine_barrier()


def _build_nc():
    nc = bass.Bass()
    inp = nc.declare_dram_parameter("inp", [H, IN_COLS], f32r, isOutput=False)
    outp = nc.declare_dram_parameter("outp", [4, 48], f32, isOutput=True)

    with SplitDrainTileContext(nc) as tc:
        with (
            tc.tile_pool(name="sb", bufs=1) as sb,
            tc.tile_pool(name="scr", bufs=3) as scr_pool,
            tc.tile_pool(name="tree", bufs=3) as tree_pool,
            tc.tile_pool(name="tail", bufs=1) as tail,
        ):
            inp_sb = sb.tile([H, IN_COLS], f32r)
            nc.sync.dma_start(inp_sb[:], inp[:])
            qT = inp_sb[:, 0:QT_COLS]
            dT = inp_sb[:, QT_COLS:QT_COLS + DT_COLS]
            ones_f = inp_sb[:, QT_COLS + DT_COLS:IN_COLS].bitcast(f32)

            # preload the exp table on ACT while the input DMA runs
            warm = tail.tile([H, 2], f32)
            nc.vector.memset(warm[:, 0:1], 0.0)
            nc.scalar.activation(
                warm[:, 1:2], warm[:, 0:1], mybir.ActivationFunctionType.Exp
            )

            maxsim = sb.tile([H, G * C_LOC], f32)  # [128, 256], written by DVE only

            with (
                tc.tile_pool(name="psA", bufs=1, space="PSUM") as psA,
                tc.tile_pool(name="psB", bufs=2, space="PSUM") as psB,
            ):
                scr = None
                scr_fill = 0
                scr_pos0 = 0
                for g in range(G):
                    lhs = qT[:, g * 128:(g + 1) * 128]
                    if g in A_GROUPS:
                        ps = psA.tile([128, 2048], f32)
                        for j in range(4):
                            nc.tensor.matmul(
                                ps[:, j * 512:(j + 1) * 512],
                                lhs, dT[:, j * 512:(j + 1) * 512],
                                start=True, stop=True,
                            )
                        col = POS[g] * C_LOC
                        nc.vector.reduce_max(
                            maxsim[:, col:col + C_LOC],
                            ps[:].rearrange("p (c d) -> p c d", d=SD),
                            axis=mybir.AxisListType.X,
                        )
                    else:
                        if scr is None:
                            scr = scr_pool.tile([128, 4 * 2048], fp16)
                            scr_fill = 0
                            scr_pos0 = POS[g]
                        for half in range(2):
                            ps = psB.tile([128, 1024], f32)
                            for j in range(2):
                                jj = half * 2 + j
                                nc.tensor.matmul(
                                    ps[:, j * 512:(j + 1) * 512],
                                    lhs, dT[:, jj * 512:(jj + 1) * 512],
                                    start=True, stop=True,
                                )
                            off = scr_fill * 2048 + half * 1024
                            nc.scalar.copy(scr[:, off:off + 1024], ps[:])
                        scr_fill += 1
                        if scr_fill == 4:
                            # fp16 pairwise max tree over d: 128 -> 64 -> 32 -> 16,
                            # then a segmented reduce 16 -> 1
                            v = scr[:].rearrange("p (s d) -> p s d", d=128)
                            t1 = tree_pool.tile([128, 4096], fp16, tag="t1")
                            nc.vector.tensor_max(
                                t1[:].rearrange("p (s d) -> p s d", d=64),
                                v[:, :, 0:64], v[:, :, 64:128],
                            )
                            v1 = t1[:].rearrange("p (s d) -> p s d", d=64)
                            t2 = tree_pool.tile([128, 2048], fp16, tag="t2")
                            nc.vector.tensor_max(
                                t2[:].rearrange("p (s d) -> p s d", d=32),
                                v1[:, :, 0:32], v1[:, :, 32:64],
                            )
                            v2 = t2[:].rearrange("p (s d) -> p s d", d=32)
                            t3 = tree_pool.tile([128, 1024], fp16, tag="t3")
                            nc.vector.tensor_max(
                                t3[:].rearrange("p (s d) -> p s d", d=16),
                                v2[:, :, 0:16], v2[:, :, 16:32],
                            )
                            col = scr_pos0 * C_LOC
                            nc.vector.reduce_max(
                                maxsim[:, col:col + 4 * C_LOC],
                                t3[:].rearrange("p (s d) -> p s d", d=16),
                                axis=mybir.AxisListType.X,
                            )
                            scr = None

            # scores[j, pos*16+c] = sum_s maxsim[(j,s), pos*16+c] for the 4
            # queries of each group: ones-matmul, plain fp32 (tiny)
            with tc.tile_pool(name="psS", bufs=1, space="PSUM") as psS:
                scores = psS.tile([4, G * C_LOC], f32)
                nc.tensor.matmul(
                    scores[:], ones_f, maxsim[:], start=True, stop=True
                )

                stage = tail.tile([4, 48], f32)
                sv = scores[:].rearrange("j (g c) -> j g c", c=C_LOC)
                # row max over the local candidates of each (j, g) row
                nc.vector.reduce_max(
                    stage[:, 0:16], sv, axis=mybir.AxisListType.X
                )
                sub = tail.tile([4, G * C_LOC], f32)
                mb = stage[:, 0:16].unsqueeze(2).broadcast_to([4, G, C_LOC])
                nc.vector.tensor_sub(
                    sub[:].rearrange("j (g c) -> j g c", c=C_LOC), sv, mb
                )
                e = tail.tile([4, G * C_LOC], f32)
                nc.scalar.activation(
                    e[:], sub[:], mybir.ActivationFunctionType.Exp, scale=K_SCALE
                )
                nc.vector.reduce_sum(
                    stage[:, 16:32],
                    e[:].rearrange("j (g c) -> j g c", c=C_LOC),
                    axis=mybir.AxisListType.X,
                )
                # local c=0 column of every group block
                nc.vector.tensor_copy(stage[:, 32:48], sv[:, :, 0:1].squeeze(2))
                nc.sync.dma_start(outp[:], stage[:])

    # This walrus build allows one sync wait per instruction. Tile emits
    # redundant same-engine WAR waits next to the covering cross-engine wait
    # (the consumer it waits on already waited on that same producer tick);
    # strip self-waits so every instruction carries a single wait.
    for f in nc.m.functions:
        for blk in f.blocks:
            for inst in blk.instructions:
                si = getattr(inst, "sync_info", None)
                if si is None or not si.on_wait or len(si.on_wait) < 2:
                    continue
                own = {u.ant_name for u in (si.on_update or [])}
                eng = str(getattr(inst, "engine", ""))
                keep = [
                    w for w in si.on_wait
                    if w.ant_name not in own
                    and not w.ant_name.startswith(f"{eng}_")
                ]
                if len(keep) != len(si.on_wait) and len(keep) <= 1:
                    si.on_wait = keep
                elif len(si.on_wait) > 1:
                    print("WARN multi-wait remains:", inst.name,
                          [w.ant_name for w in si.on_wait])
    return nc


def _prepare_inputs(q: np.ndarray, d: np.ndarray):
    qT = np.ascontiguousarray(q.transpose(2, 0, 1).reshape(H, QT_COLS))
    ones = np.zeros((H, 4), dtype=np.float32)
    for j in range(4):
        ones[j * SQ:(j + 1) * SQ, j] = 1.0
    in_maps = []
    for i in range(N_CORES):
        dT = np.ascontiguousarray(
            d[i * C_LOC:(i + 1) * C_LOC].transpose(2, 0, 1).reshape(H, DT_COLS)
        )
        inp = np.concatenate([qT, dT, ones], axis=1)
        in_maps.append({"inp": _round_f32r(inp)})
    return in_maps


def kernel(query_embeddings: np.ndarray, positive_embeddings: np.ndarray) -> np.ndarray:
    global LAST_RESULTS
    q = np.asarray(query_embeddings, dtype=np.float32)
    d = np.asarray(positive_embeddings, dtype=np.float32)
    assert q.shape == (B, SQ, H) and d.shape == (C, SD, H)

    if "nc" not in _STATE:
        _STATE["nc"] = _build_nc()
    nc = _STATE["nc"]

    in_maps = _prepare_inputs(q, d)
    res = run_bass_kernel_spmd(nc, in_maps, list(range(N_CORES)))
    LAST_RESULTS = res

    # host combine: distributed logsumexp over the 8 candidate shards
    inv_pos = np.empty(G, dtype=np.int64)
    for g, p in POS.items():
        inv_pos[g] = p
    m = np.empty((N_CORES, B), dtype=np.float64)
    s = np.empty((N_CORES, B), dtype=np.float64)
    x0 = None
    for i in range(N_CORES):
        st = np.asarray(res.results[i]["outp"], dtype=np.float64)  # [4, 48]
        mi = st[:, 0:16]     # [j, pos]
        si = st[:, 16:32]
        xi = st[:, 32:48]
        for g in range(G):
            p = inv_pos[g]
            for j in range(4):
                b = g * 4 + j
                m[i, b] = mi[j, p]
                s[i, b] = si[j, p]
        if i == 0:
            x0 = np.array([xi[b % 4, inv_pos[b // 4]] for b in range(B)])
    k = K_SCALE
    M = m.max(axis=0)                            # [B] raw-score row max
    S = (s * np.exp(k * (m - M[None, :]))).sum(axis=0)
    loss_b = np.log(S) + k * M - k * x0
    return np.float32(loss_b.mean())

